# revision 13
# baseline (speedup 1.0000x reference)
"""CURLoRA layer kernel for 8 TRN2 NeuronCores.

Computes out = x @ (W + C@U@R)^T + bias for
  x: (4, 2048, 4096) f32, W: (4096, 4096), C: (4096, 64), U: (64, 64),
  R: (64, 4096), bias: (4096,)  ->  out: (4, 2048, 4096) f32

Sharding: 8 cores = 2 token-groups x 4 output-column-groups.
Each core computes out[tg, og] = x[tg] @ (W[og] + C[og]@U@R)^T + bias[og]
independently (no collectives needed).

Per-core kernel (bf16 compute, fp32 accumulate). The host pre-stages
inputs: casts to bf16 (halves HBM reads) and packs W^T; the PE array then
runs a pure matmul stream (no PE transposes):
  1. W'^T: host-packed W^T k-groups arrive via regular DMAs on the scalar
     HWDGE queue; the adapter (C@U@R)^T = R-stationary @ (U^T C^T) is a
     K=64 matmul; DVE adds both into resident wt_sb [128d, 32k, 1024o].
  2. x^T arrives via DMA-xbar transpose ([256t, 4096] -> [128d, 32k, 256t]
     in ONE instruction) on the sync HWDGE queue, 4 chunk buffers deep.
     NOTE: concurrent dma_start_transpose on BOTH HWDGE queues corrupts
     the xbar stream - all transposes stay on sync.
  3. Main stream: per 128-token tile, 2 x 32 matmuls accumulate in PSUM;
     DVE adds bias on eviction; SWDGE (gpsimd) writes out.
"""

import sys

if "/opt/trn_rl_repo" not in sys.path:
    sys.path.insert(0, "/opt/trn_rl_repo")

import numpy as np
import ml_dtypes

B, S, D = 4, 2048, 4096
O = 4096
RK = 64
T = B * S  # 8192 tokens
NT, NO = 2, 4  # token groups x out-column groups
TSH = T // NT  # 4096 tokens per core
OSH = O // NO  # 1024 out columns per core
N_CORES = 8

NK = D // 128  # 32 k-tiles
TC = 256  # x^T chunk tokens
NCH = TSH // TC  # 16 chunks
NTT_C = TC // 128  # 2 t-tiles per chunk
NJ = OSH // 512  # 2 o-blocks of 512
NXB = 4  # x^T chunk buffers
KG = 8  # k-tiles per W load group

_CACHE = {}


def _build():
    from concourse import bacc
    import concourse.bass as bass
    import concourse.mybir as mybir
    from concourse.bass import ts
    from concourse.tile import TileContext
    from concourse.masks import make_identity

    f32 = mybir.dt.float32
    bf16 = mybir.dt.bfloat16

    nc = bacc.Bacc()
    x_ext = nc.declare_dram_parameter("x", [TSH, D], bf16, isOutput=False)
    w_ext = nc.declare_dram_parameter("W", [D, OSH], bf16, isOutput=False)
    c_ext = nc.declare_dram_parameter("C", [OSH, 128], bf16, isOutput=False)
    u_ext = nc.declare_dram_parameter("U", [RK, RK], bf16, isOutput=False)
    r_ext = nc.declare_dram_parameter("R", [RK, D], bf16, isOutput=False)
    b_ext = nc.declare_dram_parameter("bias", [OSH], f32, isOutput=False)
    out_ext = nc.declare_dram_parameter("out", [TSH, OSH], f32, isOutput=True)

    with TileContext(nc) as tc:
        with (
            tc.tile_pool(name="const", bufs=1) as const,
            tc.tile_pool(name="wt", bufs=1) as wtp,
            tc.tile_pool(name="small", bufs=1) as small,
            tc.tile_pool(name="wstage", bufs=2) as wsp,
            tc.tile_pool(name="xtpool", bufs=1) as xtpool,
            tc.tile_pool(name="opool", bufs=2) as opool,
            # PSUM: psA (ad 3 + warm 1) + psB (out 4) = 8 banks
            tc.tile_pool(name="psA", bufs=4, space="PSUM") as psA,
            tc.tile_pool(name="psB", bufs=4, space="PSUM") as psB,
        ):
            ident = const.tile([128, 128], bf16)
            make_identity(nc, ident)
            cst = const.tile([128, 512], bf16)
            for q in range(4):
                nc.vector.tensor_copy(out=cst[:, ts(q, 128)], in_=ident[:])

            # resident W'^T: [128 d-part, 32 k-tiles, 1024 o] bf16
            wt_sb = wtp.tile([128, NK, OSH], bf16)
            bias_sb = const.tile([128, OSH], f32)

            # small inputs on the SWDGE queue (gpsimd)
            u_sb = small.tile([RK, RK], bf16)
            nc.gpsimd.dma_start(out=u_sb[:], in_=u_ext[:])
            r_sb = small.tile([RK, D], bf16)
            nc.gpsimd.dma_start(out=r_sb[:], in_=r_ext[:])
            ct_sb = small.tile([128, OSH], bf16)
            nc.sync.dma_start_transpose(ct_sb[:], c_ext[:])
            ucT_sb = small.tile([RK, OSH], bf16)

            b_ap = b_ext[:]
            b_bc = bass.AP(
                tensor=b_ap.tensor,
                offset=b_ap.offset,
                ap=[[0, 128]] + [list(p) for p in b_ap.ap],
            )
            nc.gpsimd.dma_start(out=bias_sb[:], in_=b_bc)

            # x^T chunks: ONE xbar-transpose instr per chunk on sync,
            # [TC, 4096] -> [128 d, 32 k, TC]
            xt = [
                xtpool.tile([128, NK, TC], bf16, name=f"xt{b}") for b in range(NXB)
            ]

            def emit_xt_chunk(c):
                nc.sync.dma_start_transpose(xt[c % NXB][:], x_ext[ts(c, TC), :])

            for c in range(NXB):
                emit_xt_chunk(c)

            # PE warmup: keep the array streaming while first DMAs land
            # (p-state ramps to full clock after ~3us of continuous work)
            warm_ps = psA.tile([128, 512], f32, tag="w", bufs=1)
            for _ in range(16):
                nc.tensor.matmul(warm_ps[:], ident[:], cst[:], start=True, stop=True)

            # U^T C^T = (C U)^T : [64 rk, 1024 o]
            for j in range(NJ):
                ps_uc = psA.tile([128, 512], f32, tag="ad", bufs=3, name="ps_uc")
                nc.tensor.matmul(
                    ps_uc[:RK, :],
                    u_sb[:],
                    ct_sb[:RK, ts(j, 512)],
                    start=True,
                    stop=True,
                )
                nc.vector.tensor_copy(out=ucT_sb[:, ts(j, 512)], in_=ps_uc[:RK, :])

            # W'^T build: host-packed W^T k-groups via regular scalar-queue
            # DMAs; adapter (K=64 matmul vs R) added in by DVE
            for g in range(NK // KG):
                wst = wsp.tile([128, KG, OSH], bf16, name="wst")
                nc.scalar.dma_start(
                    out=wst[:],
                    in_=w_ext[ts(g, KG * 128), :].rearrange("(k p) o -> p k o", p=128),
                )
                for kk in range(KG):
                    k = g * KG + kk
                    for j in range(NJ):
                        ps_ad = psA.tile(
                            [128, 512], f32, tag="ad", bufs=3, name="ps_ad"
                        )
                        nc.tensor.matmul(
                            ps_ad[:],
                            r_sb[:, ts(k, 128)],
                            ucT_sb[:, ts(j, 512)],
                            start=True,
                            stop=True,
                        )
                        nc.vector.tensor_add(
                            out=wt_sb[:, k, ts(j, 512)],
                            in0=ps_ad[:],
                            in1=wst[:, kk, ts(j, 512)],
                        )

            # ---------------- main loop ----------------
            for c in range(NCH):
                for tt in range(NTT_C):
                    i = c * NTT_C + tt
                    out_sb = opool.tile([128, OSH], f32, name="out_sb")
                    for j in range(NJ):
                        psm = psB.tile([128, 512], f32, tag="o", bufs=4, name="psm")
                        for k in range(NK):
                            nc.tensor.matmul(
                                psm[:],
                                xt[c % NXB][:, k, ts(tt, 128)],
                                wt_sb[:, k, ts(j, 512)],
                                start=(k == 0),
                                stop=(k == NK - 1),
                            )
                        nc.vector.tensor_add(
                            out=out_sb[:, ts(j, 512)],
                            in0=psm[:],
                            in1=bias_sb[:, ts(j, 512)],
                        )
                    nc.gpsimd.dma_start(out=out_ext[ts(i, 128), :], in_=out_sb[:])
                if c + NXB < NCH:
                    emit_xt_chunk(c + NXB)

    nc.compile()
    return nc


def make_in_maps(x, W, C, U, R, bias):
    bf = ml_dtypes.bfloat16
    x = np.asarray(x, dtype=np.float32).reshape(T, D).astype(bf)
    W = np.asarray(W, dtype=np.float32).astype(bf)
    C = np.asarray(C, dtype=np.float32).astype(bf)
    C_pad = np.zeros((O, 128), dtype=bf)
    C_pad[:, :RK] = C
    U = np.ascontiguousarray(np.asarray(U, dtype=np.float32).astype(bf))
    R = np.ascontiguousarray(np.asarray(R, dtype=np.float32).astype(bf))
    bias = np.ascontiguousarray(np.asarray(bias, dtype=np.float32))

    in_maps = []
    for core in range(N_CORES):
        tg, og = divmod(core, NO)
        in_maps.append(
            {
                "x": np.ascontiguousarray(x[tg * TSH : (tg + 1) * TSH]),
                "W": np.ascontiguousarray(W[og * OSH : (og + 1) * OSH].T),
                "C": np.ascontiguousarray(C_pad[og * OSH : (og + 1) * OSH]),
                "U": U,
                "R": R,
                "bias": bias[og * OSH : (og + 1) * OSH],
            }
        )
    return in_maps


def kernel(x, W, C, U, R, bias):
    from concourse.bass_utils import run_bass_kernel_spmd

    in_maps = make_in_maps(x, W, C, U, R, bias)

    if "nc" not in _CACHE:
        _CACHE["nc"] = _build()
    nc = _CACHE["nc"]

    res = run_bass_kernel_spmd(nc, in_maps, core_ids=list(range(N_CORES)))

    out = np.empty((T, O), dtype=np.float32)
    for core in range(N_CORES):
        tg, og = divmod(core, NO)
        out[tg * TSH : (tg + 1) * TSH, og * OSH : (og + 1) * OSH] = res.results[core][
            "out"
        ]
    return out.reshape(B, S, O)


# revision 14
# speedup vs baseline: 1.1126x; 1.1126x over previous
"""CURLoRA layer kernel for 8 TRN2 NeuronCores.

Computes out = x @ (W + C@U@R)^T + bias for
  x: (4, 2048, 4096) f32, W: (4096, 4096), C: (4096, 64), U: (64, 64),
  R: (64, 4096), bias: (4096,)  ->  out: (4, 2048, 4096) f32

Sharding: 8 cores = 2 token-groups x 4 output-column-groups.
Each core computes out[tg, og] = x[tg] @ (W[og] + C[og]@U@R)^T + bias[og]
independently (no collectives needed).

Per-core kernel (bf16 compute, fp32 accumulate). The host pre-stages
inputs: casts to bf16 (halves HBM reads) and packs W^T; the PE array then
runs a pure matmul stream (no PE transposes):
  1. W'^T: host-packed W^T k-groups arrive via regular DMAs on the scalar
     HWDGE queue; the adapter (C@U@R)^T = R-stationary @ (U^T C^T) is a
     K=64 matmul; DVE adds both into resident wt_sb [128d, 32k, 1024o].
  2. x^T arrives via DMA-xbar transpose ([256t, 4096] -> [128d, 32k, 256t]
     in ONE instruction) on the sync HWDGE queue, 4 chunk buffers deep.
     NOTE: concurrent dma_start_transpose on BOTH HWDGE queues corrupts
     the xbar stream - all transposes stay on sync.
  3. Main stream: per 128-token tile, 2 x 32 matmuls accumulate in PSUM;
     DVE adds bias on eviction; SWDGE (gpsimd) writes out.
"""

import sys

if "/opt/trn_rl_repo" not in sys.path:
    sys.path.insert(0, "/opt/trn_rl_repo")

import numpy as np
import ml_dtypes

B, S, D = 4, 2048, 4096
O = 4096
RK = 64
T = B * S  # 8192 tokens
NT, NO = 2, 4  # token groups x out-column groups
TSH = T // NT  # 4096 tokens per core
OSH = O // NO  # 1024 out columns per core
N_CORES = 8

NK = D // 128  # 32 k-tiles
TC = 256  # x^T chunk tokens
NCH = TSH // TC  # 16 chunks
NTT_C = TC // 128  # 2 t-tiles per chunk
NJ = OSH // 512  # 2 o-blocks of 512
NXB = 4  # x^T chunk buffers
KG = 4  # k-tiles per W load group

_CACHE = {}


def _build():
    from concourse import bacc
    import concourse.bass as bass
    import concourse.mybir as mybir
    from concourse.bass import ts
    from concourse.tile import TileContext
    from concourse.masks import make_identity

    f32 = mybir.dt.float32
    bf16 = mybir.dt.bfloat16

    nc = bacc.Bacc()
    x_ext = nc.declare_dram_parameter("x", [TSH, D], bf16, isOutput=False)
    w_ext = nc.declare_dram_parameter("W", [D, OSH], bf16, isOutput=False)
    c_ext = nc.declare_dram_parameter("C", [OSH, 128], bf16, isOutput=False)
    u_ext = nc.declare_dram_parameter("U", [RK, RK], bf16, isOutput=False)
    r_ext = nc.declare_dram_parameter("R", [RK, D], bf16, isOutput=False)
    b_ext = nc.declare_dram_parameter("bias", [OSH], f32, isOutput=False)
    out_ext = nc.declare_dram_parameter("out", [TSH, OSH], f32, isOutput=True)

    with TileContext(nc) as tc:
        with (
            tc.tile_pool(name="const", bufs=1) as const,
            tc.tile_pool(name="wt", bufs=1) as wtp,
            tc.tile_pool(name="small", bufs=1) as small,
            tc.tile_pool(name="wstage", bufs=2) as wsp,
            tc.tile_pool(name="xtpool", bufs=1) as xtpool,
            tc.tile_pool(name="opool", bufs=2) as opool,
            # PSUM: psA (ad 3 + warm 1) + psB (out 4) = 8 banks
            tc.tile_pool(name="psA", bufs=4, space="PSUM") as psA,
            tc.tile_pool(name="psB", bufs=4, space="PSUM") as psB,
        ):
            ident = const.tile([128, 128], bf16)
            make_identity(nc, ident)
            cst = const.tile([128, 512], bf16)
            for q in range(4):
                nc.vector.tensor_copy(out=cst[:, ts(q, 128)], in_=ident[:])

            # resident W'^T: [128 d-part, 32 k-tiles, 1024 o] bf16
            wt_sb = wtp.tile([128, NK, OSH], bf16)
            bias_sb = const.tile([128, OSH], f32)

            # small inputs on the SWDGE queue (gpsimd)
            u_sb = small.tile([RK, RK], bf16)
            nc.gpsimd.dma_start(out=u_sb[:], in_=u_ext[:])
            r_sb = small.tile([RK, D], bf16)
            nc.gpsimd.dma_start(out=r_sb[:], in_=r_ext[:])
            ct_sb = small.tile([128, OSH], bf16)
            nc.sync.dma_start_transpose(ct_sb[:], c_ext[:])
            ucT_sb = small.tile([RK, OSH], bf16)

            b_ap = b_ext[:]
            b_bc = bass.AP(
                tensor=b_ap.tensor,
                offset=b_ap.offset,
                ap=[[0, 128]] + [list(p) for p in b_ap.ap],
            )
            nc.gpsimd.dma_start(out=bias_sb[:], in_=b_bc)

            # x^T chunks: ONE xbar-transpose instr per chunk on sync,
            # [TC, 4096] -> [128 d, 32 k, TC]
            xt = [
                xtpool.tile([128, NK, TC], bf16, name=f"xt{b}") for b in range(NXB)
            ]

            def emit_xt_chunk(c):
                nc.sync.dma_start_transpose(xt[c % NXB][:], x_ext[ts(c, TC), :])

            for c in range(NXB):
                emit_xt_chunk(c)

            # PE warmup: keep the array streaming while first DMAs land
            # (p-state ramps to full clock after ~3us of continuous work)
            warm_ps = psA.tile([128, 512], f32, tag="w", bufs=1)
            for _ in range(24):
                nc.tensor.matmul(warm_ps[:], ident[:], cst[:], start=True, stop=True)

            # U^T C^T = (C U)^T : [64 rk, 1024 o]
            for j in range(NJ):
                ps_uc = psA.tile([128, 512], f32, tag="ad", bufs=3, name="ps_uc")
                nc.tensor.matmul(
                    ps_uc[:RK, :],
                    u_sb[:],
                    ct_sb[:RK, ts(j, 512)],
                    start=True,
                    stop=True,
                )
                nc.vector.tensor_copy(out=ucT_sb[:, ts(j, 512)], in_=ps_uc[:RK, :])

            # W'^T build: host-packed W^T k-groups via regular scalar-queue
            # DMAs; adapter (K=64 matmul vs R) added in by DVE
            for g in range(NK // KG):
                wst = wsp.tile([128, KG, OSH], bf16, name="wst")
                nc.scalar.dma_start(
                    out=wst[:],
                    in_=w_ext[ts(g, KG * 128), :].rearrange("(k p) o -> p k o", p=128),
                )
                for kk in range(KG):
                    k = g * KG + kk
                    for j in range(NJ):
                        ps_ad = psA.tile(
                            [128, 512], f32, tag="ad", bufs=3, name="ps_ad"
                        )
                        nc.tensor.matmul(
                            ps_ad[:],
                            r_sb[:, ts(k, 128)],
                            ucT_sb[:, ts(j, 512)],
                            start=True,
                            stop=True,
                        )
                        nc.vector.tensor_add(
                            out=wt_sb[:, k, ts(j, 512)],
                            in0=ps_ad[:],
                            in1=wst[:, kk, ts(j, 512)],
                        )

            # ---------------- main loop ----------------
            for c in range(NCH):
                for tt in range(NTT_C):
                    i = c * NTT_C + tt
                    out_sb = opool.tile([128, OSH], f32, name="out_sb")
                    for j in range(NJ):
                        psm = psB.tile([128, 512], f32, tag="o", bufs=4, name="psm")
                        for k in range(NK):
                            nc.tensor.matmul(
                                psm[:],
                                xt[c % NXB][:, k, ts(tt, 128)],
                                wt_sb[:, k, ts(j, 512)],
                                start=(k == 0),
                                stop=(k == NK - 1),
                            )
                        nc.vector.tensor_add(
                            out=out_sb[:, ts(j, 512)],
                            in0=psm[:],
                            in1=bias_sb[:, ts(j, 512)],
                        )
                    nc.gpsimd.dma_start(out=out_ext[ts(i, 128), :], in_=out_sb[:])
                if c + NXB < NCH:
                    emit_xt_chunk(c + NXB)

    nc.compile()
    return nc


def make_in_maps(x, W, C, U, R, bias):
    bf = ml_dtypes.bfloat16
    x = np.asarray(x, dtype=np.float32).reshape(T, D).astype(bf)
    W = np.asarray(W, dtype=np.float32).astype(bf)
    C = np.asarray(C, dtype=np.float32).astype(bf)
    C_pad = np.zeros((O, 128), dtype=bf)
    C_pad[:, :RK] = C
    U = np.ascontiguousarray(np.asarray(U, dtype=np.float32).astype(bf))
    R = np.ascontiguousarray(np.asarray(R, dtype=np.float32).astype(bf))
    bias = np.ascontiguousarray(np.asarray(bias, dtype=np.float32))

    in_maps = []
    for core in range(N_CORES):
        tg, og = divmod(core, NO)
        in_maps.append(
            {
                "x": np.ascontiguousarray(x[tg * TSH : (tg + 1) * TSH]),
                "W": np.ascontiguousarray(W[og * OSH : (og + 1) * OSH].T),
                "C": np.ascontiguousarray(C_pad[og * OSH : (og + 1) * OSH]),
                "U": U,
                "R": R,
                "bias": bias[og * OSH : (og + 1) * OSH],
            }
        )
    return in_maps


def kernel(x, W, C, U, R, bias):
    from concourse.bass_utils import run_bass_kernel_spmd

    in_maps = make_in_maps(x, W, C, U, R, bias)

    if "nc" not in _CACHE:
        _CACHE["nc"] = _build()
    nc = _CACHE["nc"]

    res = run_bass_kernel_spmd(nc, in_maps, core_ids=list(range(N_CORES)))

    out = np.empty((T, O), dtype=np.float32)
    for core in range(N_CORES):
        tg, og = divmod(core, NO)
        out[tg * TSH : (tg + 1) * TSH, og * OSH : (og + 1) * OSH] = res.results[core][
            "out"
        ]
    return out.reshape(B, S, O)
